# revision 17
# baseline (speedup 1.0000x reference)
"""FP8 block-quantized MoE MLP (16 experts, top-4 routing) on 8 Trainium2 cores.

Strategy (expert-parallel):
  Host: build routing tables from top_k_index; each core owns 2 experts.
    For each expert, gather its routed token rows (duplicated per (token,k)
    pair), pad to a uniform capacity. Dequantize the fp8 block-quantized
    weights in exact f32 (identical arithmetic to the reference) and
    pre-transpose them to [K, O] layout, cast to the matmul dtype (f32r).
  Device (per core, per 128-row tile):
    - emulate the reference's dynamic per-token/per-128-group fp8-e4m3fn
      activation quant-dequant using the hardware fp8e4 cast with a /2
      exponent shift (TRN fp8e4 max-normal is 240 vs OCP 448; halving the
      scaled value keeps every in-range value in both formats' common
      normal range, so RNE rounds identically)
    - PE-transpose the dequantized activations, GEMM1 (8 K-blocks -> two
      PSUM banks: gate | up), silu(gate)*up, second quant-dequant, GEMM2,
      scale rows by the routing weight, DMA out.
  Host: scatter-add the weighted rows into the [2048, 1024] output.
"""

import numpy as np
import ml_dtypes

# Problem constants (hardcoded per the task contract).
T = 2048
H = 1024
I_DIM = 512
E = 16
TK = 4
G = 128
FP8_MAX = 448.0
NCORES = 8
E_LOC = E // NCORES          # experts per core
KB1 = H // 128               # 8 contraction blocks for gate_up
KB2 = I_DIM // 128           # 4 contraction blocks for down
O1 = 2 * I_DIM               # 1024
O2 = H                       # 1024

MM_DT_NAME = "float32r"

_PROGRAM_CACHE: dict = {}


def _build_program(cap: int, do_compile: bool = True):
    import concourse.bass as bass
    import concourse.mybir as mybir
    from concourse import bacc
    from concourse.tile import TileContext
    from concourse.masks import make_identity
    from contextlib import ExitStack

    dt = mybir.dt
    F32 = dt.float32
    FP8 = dt.float8e4
    # float32r containers (matmul operands) are written by DMA / ACT copies
    # only -- DVE/GPSIMD cannot touch f32r, and the BIR verifier requires
    # every producer of f32r-consumed memory to emit f32r itself.
    F32R_MODE = MM_DT_NAME == "float32r"
    MM_DT = getattr(dt, MM_DT_NAME)
    TR_DT = F32 if F32R_MODE else MM_DT   # dtype of qdq outputs + transposes
    R = E_LOC * cap
    ntiles = R // 128
    tpe = cap // 128

    nc = bacc.Bacc("TRN2")
    xg_d = nc.dram_tensor("xg", [R, H], F32, kind="ExternalInput")
    rw_d = nc.dram_tensor("rw", [R, 1], F32, kind="ExternalInput")
    BF16 = dt.bfloat16
    w1_d = nc.dram_tensor("w1", [E_LOC, 128, KB1, O1], MM_DT, kind="ExternalInput")
    w2_d = nc.dram_tensor("w2", [E_LOC, 128, KB2, O2], BF16, kind="ExternalInput")
    out_d = nc.dram_tensor("out", [R, H], F32, kind="ExternalOutput")

    AX = mybir.AxisListType.X
    OP = mybir.AluOpType
    ACT = mybir.ActivationFunctionType

    def bcast(ap, reps):
        # [P, g] -> [P, g, reps] with a stride-0 innermost dim
        return bass.AP(tensor=ap.tensor, offset=ap.offset,
                       ap=[ap.ap[0], ap.ap[1], [0, reps]])

    with TileContext(nc) as tc, ExitStack() as ctx:
        singles = ctx.enter_context(tc.tile_pool(name="singles", bufs=1))
        xpool = ctx.enter_context(tc.tile_pool(name="xpool", bufs=4))
        rwpool = ctx.enter_context(tc.tile_pool(name="rwpool", bufs=3))
        spool = ctx.enter_context(tc.tile_pool(name="spool", bufs=8))
        qpool = ctx.enter_context(tc.tile_pool(name="qpool", bufs=3))
        dqpool = ctx.enter_context(tc.tile_pool(name="dqpool", bufs=3))
        tpool = ctx.enter_context(tc.tile_pool(name="tpool", bufs=3))
        hpool = ctx.enter_context(tc.tile_pool(name="hpool", bufs=3))
        opool = ctx.enter_context(tc.tile_pool(name="opool", bufs=3))
        ps_tx = ctx.enter_context(tc.tile_pool(name="ps_tx", bufs=2, space="PSUM"))
        ps_m1 = ctx.enter_context(tc.tile_pool(name="ps_m1", bufs=2, space="PSUM"))
        ps_m2 = ctx.enter_context(tc.tile_pool(name="ps_m2", bufs=1, space="PSUM"))

        ident = singles.tile([128, 128], TR_DT)
        make_identity(nc, ident)

        w1_sb = singles.tile([128, E_LOC, KB1, O1], MM_DT)
        w2_sb = singles.tile([128, E_LOC, KB2, O2], BF16)
        ident_bf = singles.tile([128, 128], BF16)
        make_identity(nc, ident_bf)
        for e in range(E_LOC):
            for kb in range(KB1):
                nc.sync.dma_start(out=w1_sb[:, e, kb], in_=w1_d[e, :, kb])
            for kb in range(KB2):
                nc.sync.dma_start(out=w2_sb[:, e, kb], in_=w2_d[e, :, kb])

        # HAM warmup: ~4us of back-to-back PE work so the clock gate opens
        # before (and stays open through) the weight-DMA-paced first tiles.
        warm_ps = ps_m2.tile([128, 1024], F32, tag="ps_o")
        for _ in range(16):
            nc.tensor.transpose(warm_ps[:, 0:128], ident, ident)

        # Software pipeline: per-engine instruction queues execute in emission
        # order, so emit x-side work two tiles ahead and h-side one behind:
        #   PE order per step: tx_i, gemm1_i, txh_{i-1}, gemm2_{i-1}
        #   DVE order per step: h_{i-1}, x_{i+2}
        # keeping every engine dense instead of serializing tile-by-tile.
        state = {}

        def emit_x(it):
            """load + quant-dequant (emulates fp8 e4m3fn) for tile `it`.
            scl2 = 2*(max(amax,1e-10)/448); q8 = fp8(x/scl2) equals
            e4m3fn(x/scale)/2 for all normal values."""
            r0 = it * 128
            x_t = xpool.tile([128, H], F32)
            nc.sync.dma_start(out=x_t, in_=xg_d[r0:r0 + 128, :])
            rw_t = rwpool.tile([128, 1], F32)
            nc.sync.dma_start(out=rw_t, in_=rw_d[r0:r0 + 128, :])
            amax = spool.tile([128, KB1], F32, tag="amax_x")
            nc.vector.tensor_reduce(
                out=amax, in_=x_t.rearrange("p (g j) -> p g j", j=128),
                axis=AX, op=OP.max, apply_absolute_value=True)
            scl2 = spool.tile([128, KB1], F32, tag="scl_x")
            nc.vector.tensor_scalar(out=scl2, in0=amax, scalar1=1e-10,
                                    scalar2=2.0 / FP8_MAX, op0=OP.max,
                                    op1=OP.mult)
            inv2 = spool.tile([128, KB1], F32, tag="inv_x")
            nc.vector.reciprocal(inv2, scl2)
            q8 = qpool.tile([128, KB1, 128], FP8, tag="q8_x")
            nc.vector.tensor_tensor(
                out=q8, in0=x_t.rearrange("p (g j) -> p g j", j=128),
                in1=bcast(inv2, 128), op=OP.mult)
            xq = dqpool.tile([128, KB1, 128], TR_DT, tag="xq")
            nc.gpsimd.tensor_tensor(out=xq, in0=q8, in1=bcast(scl2, 128),
                                    op=OP.mult)
            state[it] = {"xq": xq, "rw": rw_t}

        def emit_tx_g1(it):
            """transpose xq + GEMM1 for tile `it`."""
            e = it // tpe
            st = state[it]
            xq = st["xq"]
            xqT = tpool.tile([128, KB1, 128], MM_DT, tag="xqT")
            for half in range(2):
                pst = ps_tx.tile([128, 4, 128], TR_DT, tag="ps_tx")
                for j in range(4):
                    kb = half * 4 + j
                    nc.tensor.transpose(pst[:, j], xq[:, kb, :], ident)
                nc.scalar.copy(out=xqT[:, half * 4:(half + 1) * 4, :], in_=pst)
            ps_gu = ps_m1.tile([128, 1024], F32, tag="ps_gu")
            for kb in range(KB1):
                nc.tensor.matmul(ps_gu[:, 0:512], xqT[:, kb, :],
                                 w1_sb[:, e, kb, 0:512],
                                 start=(kb == 0), stop=(kb == KB1 - 1))
                nc.tensor.matmul(ps_gu[:, 512:1024], xqT[:, kb, :],
                                 w1_sb[:, e, kb, 512:1024],
                                 start=(kb == 0), stop=(kb == KB1 - 1))
            st["gu"] = ps_gu

        def emit_h_g2(it):
            """silu/quant + transpose + GEMM2 + weighted output for tile `it`."""
            e = it // tpe
            r0 = it * 128
            st = state.pop(it)
            ps_gu = st["gu"]
            ps_gate = ps_gu[:, 0:512]
            ps_up = ps_gu[:, 512:1024]
            h_t = hpool.tile([128, I_DIM], F32, tag="h")
            nc.scalar.activation(out=h_t, in_=ps_gate, func=ACT.Silu)
            nc.vector.tensor_tensor(out=h_t, in0=h_t, in1=ps_up, op=OP.mult)
            amax_h = spool.tile([128, KB2], F32, tag="amax_h")
            nc.vector.tensor_reduce(
                out=amax_h, in_=h_t.rearrange("p (g j) -> p g j", j=128),
                axis=AX, op=OP.max, apply_absolute_value=True)
            scl2h = spool.tile([128, KB2], F32, tag="scl_h")
            nc.vector.tensor_scalar(out=scl2h, in0=amax_h, scalar1=1e-10,
                                    scalar2=2.0 / FP8_MAX, op0=OP.max,
                                    op1=OP.mult)
            inv2h = spool.tile([128, KB2], F32, tag="inv_h")
            nc.vector.reciprocal(inv2h, scl2h)
            q8h = qpool.tile([128, KB2, 128], FP8, tag="q8_h")
            nc.vector.tensor_tensor(
                out=q8h, in0=h_t.rearrange("p (g j) -> p g j", j=128),
                in1=bcast(inv2h, 128), op=OP.mult)
            hq = dqpool.tile([128, KB2, 128], BF16, tag="hq")
            nc.gpsimd.tensor_tensor(out=hq, in0=q8h, in1=bcast(scl2h, 128),
                                    op=OP.mult)
            hT = tpool.tile([128, KB2, 128], BF16, tag="hT")
            psh = ps_tx.tile([128, 4, 128], BF16, tag="ps_tx")
            for kb in range(KB2):
                nc.tensor.transpose(psh[:, kb], hq[:, kb, :], ident_bf)
            nc.scalar.copy(out=hT, in_=psh)
            ps_o = ps_m2.tile([128, 1024], F32, tag="ps_o")
            for kb in range(KB2):
                nc.tensor.matmul(ps_o[:, 0:512], hT[:, kb, :],
                                 w2_sb[:, e, kb, 0:512],
                                 start=(kb == 0), stop=(kb == KB2 - 1))
                nc.tensor.matmul(ps_o[:, 512:1024], hT[:, kb, :],
                                 w2_sb[:, e, kb, 512:1024],
                                 start=(kb == 0), stop=(kb == KB2 - 1))
            o_t = opool.tile([128, H], F32, tag="o")
            nc.scalar.activation(out=o_t[:, 0:512], in_=ps_o[:, 0:512],
                                 func=ACT.Copy, scale=st["rw"])
            nc.scalar.activation(out=o_t[:, 512:1024], in_=ps_o[:, 512:1024],
                                 func=ACT.Copy, scale=st["rw"])
            nc.sync.dma_start(out=out_d[r0:r0 + 128, :], in_=o_t)

        emit_x(0)
        if ntiles > 1:
            emit_x(1)
        for it in range(ntiles):
            emit_tx_g1(it)
            if it >= 1:
                emit_h_g2(it - 1)
            if it + 2 < ntiles:
                emit_x(it + 2)
        emit_h_g2(ntiles - 1)

    if do_compile:
        nc.compile()
    return nc


def _get_program(cap: int):
    key = (cap, MM_DT_NAME)
    if key not in _PROGRAM_CACHE:
        _PROGRAM_CACHE[key] = _build_program(cap)
    return _PROGRAM_CACHE[key]


def _dequant_weight(w, s, g=G):
    E_, O_, K_ = w.shape
    wb = w.reshape(E_, O_ // g, g, K_ // g, g)
    return (wb * s[:, :, None, :, None]).reshape(E_, O_, K_)


def _prep(inputs):
    hs = np.ascontiguousarray(np.asarray(inputs["hidden_states"], np.float32))
    idx = np.asarray(inputs["top_k_index"]).astype(np.int64)
    tkw = np.asarray(inputs["top_k_weights"], np.float32)
    gup = np.asarray(inputs["gate_up_proj"], np.float32)
    gup_s = np.asarray(inputs["gate_up_proj_scale_inv"], np.float32)
    dn = np.asarray(inputs["down_proj"], np.float32)
    dn_s = np.asarray(inputs["down_proj_scale_inv"], np.float32)

    # routing tables: merge duplicate (token, expert) pairs (the reference
    # sums top-k weights per expert), then group by expert
    flat_e = idx.reshape(-1)
    flat_t = np.repeat(np.arange(T, dtype=np.int64), TK)
    flat_w = tkw.reshape(-1).astype(np.float64)
    key = flat_e * T + flat_t
    uk, inv = np.unique(key, return_inverse=True)
    sw = np.bincount(inv, weights=flat_w).astype(np.float32)
    se = (uk // T).astype(np.int64)
    st = (uk % T).astype(np.int64)
    counts = np.bincount(se, minlength=E)
    cap = int(np.ceil(max(int(counts.max()), 1) / 128.0) * 128)
    R = E_LOC * cap

    starts = np.zeros(E + 1, np.int64)
    np.cumsum(counts, out=starts[1:])

    # exact f32 dequant (same arithmetic as the reference), -> [K, O] layout
    w1_full = _dequant_weight(gup, gup_s)            # [E, O1, H]
    w2_full = _dequant_weight(dn, dn_s)              # [E, O2, I]
    mmdt = ml_dtypes.bfloat16 if MM_DT_NAME == "bfloat16" else np.float32
    # [E, K, O] -> [E, 128, KB, O]
    w1_t = np.ascontiguousarray(
        w1_full.transpose(0, 2, 1).reshape(E, KB1, 128, O1).transpose(0, 2, 1, 3)
    ).astype(mmdt)
    w2_t = np.ascontiguousarray(
        w2_full.transpose(0, 2, 1).reshape(E, KB2, 128, O2).transpose(0, 2, 1, 3)
    ).astype(ml_dtypes.bfloat16)

    in_maps = []
    tok_core = []      # per-core valid token ids (concatenated per expert)
    nvalid_core = []   # per-core list of (row_offset, count)
    for c in range(NCORES):
        rows_idx = np.zeros(R, np.int64)
        rw_vec = np.zeros(R, np.float32)
        segs = []
        for j in range(E_LOC):
            e = c * E_LOC + j
            n = int(counts[e])
            s0, r0 = starts[e], j * cap
            rows_idx[r0:r0 + n] = st[s0:s0 + n]
            rw_vec[r0:r0 + n] = sw[s0:s0 + n]
            segs.append((r0, n))
        xg = hs[rows_idx]  # [R, H]
        in_maps.append({
            "xg": xg,
            "rw": rw_vec.reshape(R, 1),
            "w1": np.ascontiguousarray(w1_t[c * E_LOC:(c + 1) * E_LOC]),
            "w2": np.ascontiguousarray(w2_t[c * E_LOC:(c + 1) * E_LOC]),
        })
        tok_core.append(rows_idx)
        nvalid_core.append(segs)
    return cap, in_maps, tok_core, nvalid_core


def _combine(results, tok_core, nvalid_core):
    out = np.zeros((T, H), np.float32)
    for c in range(NCORES):
        res = results[c]["out"]
        for (r0, n) in nvalid_core[c]:
            if n:
                np.add.at(out, tok_core[c][r0:r0 + n], res[r0:r0 + n])
    return out


def kernel_with_results(inputs, trace=False):
    from concourse.bass_utils import run_bass_kernel_spmd
    cap, in_maps, tok_core, nvalid_core = _prep(inputs)
    nc = _get_program(cap)
    bres = run_bass_kernel_spmd(nc, in_maps, core_ids=list(range(NCORES)),
                                trace=trace)
    out = _combine(bres.results, tok_core, nvalid_core)
    return out, bres


def kernel(**inputs) -> np.ndarray:
    out, _ = kernel_with_results(inputs, trace=False)
    return out


# revision 18
# speedup vs baseline: 1.1776x; 1.1776x over previous
"""FP8 block-quantized MoE MLP (16 experts, top-4 routing) on 8 Trainium2 cores.

Strategy (expert-parallel):
  Host: build routing tables from top_k_index; each core owns 2 experts.
    For each expert, gather its routed token rows (duplicated per (token,k)
    pair), pad to a uniform capacity. Dequantize the fp8 block-quantized
    weights in exact f32 (identical arithmetic to the reference) and
    pre-transpose them to [K, O] layout, cast to the matmul dtype (f32r).
  Device (per core, per 128-row tile):
    - emulate the reference's dynamic per-token/per-128-group fp8-e4m3fn
      activation quant-dequant using the hardware fp8e4 cast with a /2
      exponent shift (TRN fp8e4 max-normal is 240 vs OCP 448; halving the
      scaled value keeps every in-range value in both formats' common
      normal range, so RNE rounds identically)
    - PE-transpose the dequantized activations, GEMM1 (8 K-blocks -> two
      PSUM banks: gate | up), silu(gate)*up, second quant-dequant, GEMM2,
      scale rows by the routing weight, DMA out.
  Host: scatter-add the weighted rows into the [2048, 1024] output.
"""

import numpy as np
import ml_dtypes

# Problem constants (hardcoded per the task contract).
T = 2048
H = 1024
I_DIM = 512
E = 16
TK = 4
G = 128
FP8_MAX = 448.0
NCORES = 8
E_LOC = E // NCORES          # experts per core
KB1 = H // 128               # 8 contraction blocks for gate_up
KB2 = I_DIM // 128           # 4 contraction blocks for down
O1 = 2 * I_DIM               # 1024
O2 = H                       # 1024

MM_DT_NAME = "float32r"

_PROGRAM_CACHE: dict = {}


def _build_program(cap: int, do_compile: bool = True):
    import concourse.bass as bass
    import concourse.mybir as mybir
    from concourse import bacc
    from concourse.tile import TileContext
    from concourse.masks import make_identity
    from contextlib import ExitStack

    dt = mybir.dt
    F32 = dt.float32
    FP8 = dt.float8e4
    # float32r containers (matmul operands) are written by DMA / ACT copies
    # only -- DVE/GPSIMD cannot touch f32r, and the BIR verifier requires
    # every producer of f32r-consumed memory to emit f32r itself.
    F32R_MODE = MM_DT_NAME == "float32r"
    MM_DT = getattr(dt, MM_DT_NAME)
    TR_DT = F32 if F32R_MODE else MM_DT   # dtype of qdq outputs + transposes
    R = E_LOC * cap
    ntiles = R // 128
    tpe = cap // 128

    nc = bacc.Bacc("TRN2")
    xg_d = nc.dram_tensor("xg", [R, H], F32, kind="ExternalInput")
    rw_d = nc.dram_tensor("rw", [R, 1], F32, kind="ExternalInput")
    BF16 = dt.bfloat16
    w1_d = nc.dram_tensor("w1", [E_LOC, 128, KB1, O1], MM_DT, kind="ExternalInput")
    w2_d = nc.dram_tensor("w2", [E_LOC, 128, KB2, O2], BF16, kind="ExternalInput")
    out_d = nc.dram_tensor("out", [R, H], F32, kind="ExternalOutput")

    AX = mybir.AxisListType.X
    OP = mybir.AluOpType
    ACT = mybir.ActivationFunctionType

    def bcast(ap, reps):
        # [P, g] -> [P, g, reps] with a stride-0 innermost dim
        return bass.AP(tensor=ap.tensor, offset=ap.offset,
                       ap=[ap.ap[0], ap.ap[1], [0, reps]])

    with TileContext(nc) as tc, ExitStack() as ctx:
        singles = ctx.enter_context(tc.tile_pool(name="singles", bufs=1))
        xpool = ctx.enter_context(tc.tile_pool(name="xpool", bufs=4))
        rwpool = ctx.enter_context(tc.tile_pool(name="rwpool", bufs=6))
        spool = ctx.enter_context(tc.tile_pool(name="spool", bufs=8))
        qpool = ctx.enter_context(tc.tile_pool(name="qpool", bufs=3))
        dqpool = ctx.enter_context(tc.tile_pool(name="dqpool", bufs=3))
        tpool = ctx.enter_context(tc.tile_pool(name="tpool", bufs=3))
        hpool = ctx.enter_context(tc.tile_pool(name="hpool", bufs=3))
        opool = ctx.enter_context(tc.tile_pool(name="opool", bufs=3))
        ps_tx = ctx.enter_context(tc.tile_pool(name="ps_tx", bufs=2, space="PSUM"))
        ps_m1 = ctx.enter_context(tc.tile_pool(name="ps_m1", bufs=2, space="PSUM"))
        ps_m2 = ctx.enter_context(tc.tile_pool(name="ps_m2", bufs=1, space="PSUM"))

        ident = singles.tile([128, 128], TR_DT)
        make_identity(nc, ident)

        w1_sb = singles.tile([128, E_LOC, KB1, O1], MM_DT)
        w2_sb = singles.tile([128, E_LOC, KB2, O2], BF16)
        ident_bf = singles.tile([128, 128], BF16)
        make_identity(nc, ident_bf)
        for e in range(E_LOC):
            for kb in range(KB1):
                nc.sync.dma_start(out=w1_sb[:, e, kb], in_=w1_d[e, :, kb])
            for kb in range(KB2):
                nc.sync.dma_start(out=w2_sb[:, e, kb], in_=w2_d[e, :, kb])

        # HAM warmup: ~4us of back-to-back PE work so the clock gate opens
        # before (and stays open through) the weight-DMA-paced first tiles.
        warm_ps = ps_m2.tile([128, 1024], F32, tag="ps_o")
        for _ in range(16):
            nc.tensor.transpose(warm_ps[:, 0:128], ident, ident)

        # Software pipeline: per-engine instruction queues execute in emission
        # order, so emit x-side work two tiles ahead and h-side one behind:
        #   PE order per step: tx_i, gemm1_i, txh_{i-1}, gemm2_{i-1}
        #   DVE order per step: h_{i-1}, x_{i+2}
        # keeping every engine dense instead of serializing tile-by-tile.
        state = {}

        def emit_x(it):
            """load + quant-dequant (emulates fp8 e4m3fn) for tile `it`.
            scl2 = 2*(max(amax,1e-10)/448); q8 = fp8(x/scl2) equals
            e4m3fn(x/scale)/2 for all normal values."""
            r0 = it * 128
            x_t = xpool.tile([128, H], F32)
            nc.gpsimd.dma_start(out=x_t, in_=xg_d[r0:r0 + 128, :])
            rw_t = rwpool.tile([128, 1], F32)
            nc.gpsimd.dma_start(out=rw_t, in_=rw_d[r0:r0 + 128, :])
            amax = spool.tile([128, KB1], F32, tag="amax_x")
            nc.vector.tensor_reduce(
                out=amax, in_=x_t.rearrange("p (g j) -> p g j", j=128),
                axis=AX, op=OP.max, apply_absolute_value=True)
            scl2 = spool.tile([128, KB1], F32, tag="scl_x")
            nc.vector.tensor_scalar(out=scl2, in0=amax, scalar1=1e-10,
                                    scalar2=2.0 / FP8_MAX, op0=OP.max,
                                    op1=OP.mult)
            inv2 = spool.tile([128, KB1], F32, tag="inv_x")
            nc.vector.reciprocal(inv2, scl2)
            q8 = qpool.tile([128, KB1, 128], FP8, tag="q8_x")
            nc.vector.tensor_tensor(
                out=q8, in0=x_t.rearrange("p (g j) -> p g j", j=128),
                in1=bcast(inv2, 128), op=OP.mult)
            xq = dqpool.tile([128, KB1, 128], TR_DT, tag="xq")
            nc.gpsimd.tensor_tensor(out=xq, in0=q8, in1=bcast(scl2, 128),
                                    op=OP.mult)
            state[it] = {"xq": xq, "rw": rw_t}

        def emit_tx_g1(it):
            """transpose xq + GEMM1 for tile `it`."""
            e = it // tpe
            st = state[it]
            xq = st["xq"]
            xqT = tpool.tile([128, KB1, 128], MM_DT, tag="xqT")
            for half in range(2):
                pst = ps_tx.tile([128, 4, 128], TR_DT, tag="ps_tx")
                for j in range(4):
                    kb = half * 4 + j
                    nc.tensor.transpose(pst[:, j], xq[:, kb, :], ident)
                nc.scalar.copy(out=xqT[:, half * 4:(half + 1) * 4, :], in_=pst)
            ps_gu = ps_m1.tile([128, 1024], F32, tag="ps_gu")
            for kb in range(KB1):
                nc.tensor.matmul(ps_gu[:, 0:512], xqT[:, kb, :],
                                 w1_sb[:, e, kb, 0:512],
                                 start=(kb == 0), stop=(kb == KB1 - 1))
                nc.tensor.matmul(ps_gu[:, 512:1024], xqT[:, kb, :],
                                 w1_sb[:, e, kb, 512:1024],
                                 start=(kb == 0), stop=(kb == KB1 - 1))
            st["gu"] = ps_gu

        def emit_h_g2(it):
            """silu/quant + transpose + GEMM2 + weighted output for tile `it`."""
            e = it // tpe
            r0 = it * 128
            st = state.pop(it)
            ps_gu = st["gu"]
            ps_gate = ps_gu[:, 0:512]
            ps_up = ps_gu[:, 512:1024]
            h_t = hpool.tile([128, I_DIM], F32, tag="h")
            nc.scalar.activation(out=h_t, in_=ps_gate, func=ACT.Silu)
            nc.vector.tensor_tensor(out=h_t, in0=h_t, in1=ps_up, op=OP.mult)
            amax_h = spool.tile([128, KB2], F32, tag="amax_h")
            nc.vector.tensor_reduce(
                out=amax_h, in_=h_t.rearrange("p (g j) -> p g j", j=128),
                axis=AX, op=OP.max, apply_absolute_value=True)
            scl2h = spool.tile([128, KB2], F32, tag="scl_h")
            nc.vector.tensor_scalar(out=scl2h, in0=amax_h, scalar1=1e-10,
                                    scalar2=2.0 / FP8_MAX, op0=OP.max,
                                    op1=OP.mult)
            inv2h = spool.tile([128, KB2], F32, tag="inv_h")
            nc.vector.reciprocal(inv2h, scl2h)
            q8h = qpool.tile([128, KB2, 128], FP8, tag="q8_h")
            nc.vector.tensor_tensor(
                out=q8h, in0=h_t.rearrange("p (g j) -> p g j", j=128),
                in1=bcast(inv2h, 128), op=OP.mult)
            hq = dqpool.tile([128, KB2, 128], BF16, tag="hq")
            nc.gpsimd.tensor_tensor(out=hq, in0=q8h, in1=bcast(scl2h, 128),
                                    op=OP.mult)
            hT = tpool.tile([128, KB2, 128], BF16, tag="hT")
            psh = ps_tx.tile([128, 4, 128], BF16, tag="ps_tx")
            for kb in range(KB2):
                nc.tensor.transpose(psh[:, kb], hq[:, kb, :], ident_bf)
            nc.scalar.copy(out=hT, in_=psh)
            ps_o = ps_m2.tile([128, 1024], F32, tag="ps_o")
            for kb in range(KB2):
                nc.tensor.matmul(ps_o[:, 0:512], hT[:, kb, :],
                                 w2_sb[:, e, kb, 0:512],
                                 start=(kb == 0), stop=(kb == KB2 - 1))
                nc.tensor.matmul(ps_o[:, 512:1024], hT[:, kb, :],
                                 w2_sb[:, e, kb, 512:1024],
                                 start=(kb == 0), stop=(kb == KB2 - 1))
            o_t = opool.tile([128, H], F32, tag="o")
            nc.scalar.activation(out=o_t[:, 0:512], in_=ps_o[:, 0:512],
                                 func=ACT.Copy, scale=st["rw"])
            nc.scalar.activation(out=o_t[:, 512:1024], in_=ps_o[:, 512:1024],
                                 func=ACT.Copy, scale=st["rw"])
            nc.gpsimd.dma_start(out=out_d[r0:r0 + 128, :], in_=o_t)

        emit_x(0)
        if ntiles > 1:
            emit_x(1)
        for it in range(ntiles):
            emit_tx_g1(it)
            if it >= 1:
                emit_h_g2(it - 1)
            if it + 2 < ntiles:
                emit_x(it + 2)
        emit_h_g2(ntiles - 1)

    if do_compile:
        nc.compile()
    return nc


def _get_program(cap: int):
    key = (cap, MM_DT_NAME)
    if key not in _PROGRAM_CACHE:
        _PROGRAM_CACHE[key] = _build_program(cap)
    return _PROGRAM_CACHE[key]


def _dequant_weight(w, s, g=G):
    E_, O_, K_ = w.shape
    wb = w.reshape(E_, O_ // g, g, K_ // g, g)
    return (wb * s[:, :, None, :, None]).reshape(E_, O_, K_)


def _prep(inputs):
    hs = np.ascontiguousarray(np.asarray(inputs["hidden_states"], np.float32))
    idx = np.asarray(inputs["top_k_index"]).astype(np.int64)
    tkw = np.asarray(inputs["top_k_weights"], np.float32)
    gup = np.asarray(inputs["gate_up_proj"], np.float32)
    gup_s = np.asarray(inputs["gate_up_proj_scale_inv"], np.float32)
    dn = np.asarray(inputs["down_proj"], np.float32)
    dn_s = np.asarray(inputs["down_proj_scale_inv"], np.float32)

    # routing tables: merge duplicate (token, expert) pairs (the reference
    # sums top-k weights per expert), then group by expert
    flat_e = idx.reshape(-1)
    flat_t = np.repeat(np.arange(T, dtype=np.int64), TK)
    flat_w = tkw.reshape(-1).astype(np.float64)
    key = flat_e * T + flat_t
    uk, inv = np.unique(key, return_inverse=True)
    sw = np.bincount(inv, weights=flat_w).astype(np.float32)
    se = (uk // T).astype(np.int64)
    st = (uk % T).astype(np.int64)
    counts = np.bincount(se, minlength=E)
    cap = int(np.ceil(max(int(counts.max()), 1) / 128.0) * 128)
    R = E_LOC * cap

    starts = np.zeros(E + 1, np.int64)
    np.cumsum(counts, out=starts[1:])

    # exact f32 dequant (same arithmetic as the reference), -> [K, O] layout
    w1_full = _dequant_weight(gup, gup_s)            # [E, O1, H]
    w2_full = _dequant_weight(dn, dn_s)              # [E, O2, I]
    mmdt = ml_dtypes.bfloat16 if MM_DT_NAME == "bfloat16" else np.float32
    # [E, K, O] -> [E, 128, KB, O]
    w1_t = np.ascontiguousarray(
        w1_full.transpose(0, 2, 1).reshape(E, KB1, 128, O1).transpose(0, 2, 1, 3)
    ).astype(mmdt)
    w2_t = np.ascontiguousarray(
        w2_full.transpose(0, 2, 1).reshape(E, KB2, 128, O2).transpose(0, 2, 1, 3)
    ).astype(ml_dtypes.bfloat16)

    in_maps = []
    tok_core = []      # per-core valid token ids (concatenated per expert)
    nvalid_core = []   # per-core list of (row_offset, count)
    for c in range(NCORES):
        rows_idx = np.zeros(R, np.int64)
        rw_vec = np.zeros(R, np.float32)
        segs = []
        for j in range(E_LOC):
            e = c * E_LOC + j
            n = int(counts[e])
            s0, r0 = starts[e], j * cap
            rows_idx[r0:r0 + n] = st[s0:s0 + n]
            rw_vec[r0:r0 + n] = sw[s0:s0 + n]
            segs.append((r0, n))
        xg = hs[rows_idx]  # [R, H]
        in_maps.append({
            "xg": xg,
            "rw": rw_vec.reshape(R, 1),
            "w1": np.ascontiguousarray(w1_t[c * E_LOC:(c + 1) * E_LOC]),
            "w2": np.ascontiguousarray(w2_t[c * E_LOC:(c + 1) * E_LOC]),
        })
        tok_core.append(rows_idx)
        nvalid_core.append(segs)
    return cap, in_maps, tok_core, nvalid_core


def _combine(results, tok_core, nvalid_core):
    out = np.zeros((T, H), np.float32)
    for c in range(NCORES):
        res = results[c]["out"]
        for (r0, n) in nvalid_core[c]:
            if n:
                np.add.at(out, tok_core[c][r0:r0 + n], res[r0:r0 + n])
    return out


def kernel_with_results(inputs, trace=False):
    from concourse.bass_utils import run_bass_kernel_spmd
    cap, in_maps, tok_core, nvalid_core = _prep(inputs)
    nc = _get_program(cap)
    bres = run_bass_kernel_spmd(nc, in_maps, core_ids=list(range(NCORES)),
                                trace=trace)
    out = _combine(bres.results, tok_core, nvalid_core)
    return out, bres


def kernel(**inputs) -> np.ndarray:
    out, _ = kernel_with_results(inputs, trace=False)
    return out


# revision 19
# speedup vs baseline: 1.1779x; 1.0003x over previous
"""FP8 block-quantized MoE MLP (16 experts, top-4 routing) on 8 Trainium2 cores.

Strategy (expert-parallel):
  Host: build routing tables from top_k_index; each core owns 2 experts.
    For each expert, gather its routed token rows (duplicated per (token,k)
    pair), pad to a uniform capacity. Dequantize the fp8 block-quantized
    weights in exact f32 (identical arithmetic to the reference) and
    pre-transpose them to [K, O] layout, cast to the matmul dtype (f32r).
  Device (per core, per 128-row tile):
    - emulate the reference's dynamic per-token/per-128-group fp8-e4m3fn
      activation quant-dequant using the hardware fp8e4 cast with a /2
      exponent shift (TRN fp8e4 max-normal is 240 vs OCP 448; halving the
      scaled value keeps every in-range value in both formats' common
      normal range, so RNE rounds identically)
    - PE-transpose the dequantized activations, GEMM1 (8 K-blocks -> two
      PSUM banks: gate | up), silu(gate)*up, second quant-dequant, GEMM2,
      scale rows by the routing weight, DMA out.
  Host: scatter-add the weighted rows into the [2048, 1024] output.
"""

import numpy as np
import ml_dtypes

# Problem constants (hardcoded per the task contract).
T = 2048
H = 1024
I_DIM = 512
E = 16
TK = 4
G = 128
FP8_MAX = 448.0
NCORES = 8
E_LOC = E // NCORES          # experts per core
KB1 = H // 128               # 8 contraction blocks for gate_up
KB2 = I_DIM // 128           # 4 contraction blocks for down
O1 = 2 * I_DIM               # 1024
O2 = H                       # 1024

MM_DT_NAME = "float32r"

_PROGRAM_CACHE: dict = {}


def _build_program(cap: int, do_compile: bool = True):
    import concourse.bass as bass
    import concourse.mybir as mybir
    from concourse import bacc
    from concourse.tile import TileContext
    from concourse.masks import make_identity
    from contextlib import ExitStack

    dt = mybir.dt
    F32 = dt.float32
    FP8 = dt.float8e4
    # float32r containers (matmul operands) are written by DMA / ACT copies
    # only -- DVE/GPSIMD cannot touch f32r, and the BIR verifier requires
    # every producer of f32r-consumed memory to emit f32r itself.
    F32R_MODE = MM_DT_NAME == "float32r"
    MM_DT = getattr(dt, MM_DT_NAME)
    TR_DT = F32 if F32R_MODE else MM_DT   # dtype of qdq outputs + transposes
    R = E_LOC * cap
    ntiles = R // 128
    tpe = cap // 128

    nc = bacc.Bacc("TRN2")
    xg_d = nc.dram_tensor("xg", [R, H], F32, kind="ExternalInput")
    rw_d = nc.dram_tensor("rw", [R, 1], F32, kind="ExternalInput")
    BF16 = dt.bfloat16
    w1_d = nc.dram_tensor("w1", [E_LOC, 128, KB1, O1], MM_DT, kind="ExternalInput")
    w2_d = nc.dram_tensor("w2", [E_LOC, 128, KB2, O2], BF16, kind="ExternalInput")
    out_d = nc.dram_tensor("out", [R, H], F32, kind="ExternalOutput")

    AX = mybir.AxisListType.X
    OP = mybir.AluOpType
    ACT = mybir.ActivationFunctionType

    def bcast(ap, reps):
        # [P, g] -> [P, g, reps] with a stride-0 innermost dim
        return bass.AP(tensor=ap.tensor, offset=ap.offset,
                       ap=[ap.ap[0], ap.ap[1], [0, reps]])

    with TileContext(nc) as tc, ExitStack() as ctx:
        singles = ctx.enter_context(tc.tile_pool(name="singles", bufs=1))
        xpool = ctx.enter_context(tc.tile_pool(name="xpool", bufs=5))
        rwpool = ctx.enter_context(tc.tile_pool(name="rwpool", bufs=7))
        spool = ctx.enter_context(tc.tile_pool(name="spool", bufs=10))
        qpool = ctx.enter_context(tc.tile_pool(name="qpool", bufs=4))
        dqpool = ctx.enter_context(tc.tile_pool(name="dqpool", bufs=4))
        tpool = ctx.enter_context(tc.tile_pool(name="tpool", bufs=3))
        hpool = ctx.enter_context(tc.tile_pool(name="hpool", bufs=3))
        opool = ctx.enter_context(tc.tile_pool(name="opool", bufs=3))
        ps_tx = ctx.enter_context(tc.tile_pool(name="ps_tx", bufs=2, space="PSUM"))
        ps_m1 = ctx.enter_context(tc.tile_pool(name="ps_m1", bufs=2, space="PSUM"))
        ps_m2 = ctx.enter_context(tc.tile_pool(name="ps_m2", bufs=1, space="PSUM"))

        ident = singles.tile([128, 128], TR_DT)
        make_identity(nc, ident)

        w1_sb = singles.tile([128, E_LOC, KB1, O1], MM_DT)
        w2_sb = singles.tile([128, E_LOC, KB2, O2], BF16)
        ident_bf = singles.tile([128, 128], BF16)
        make_identity(nc, ident_bf)
        for e in range(E_LOC):
            for kb in range(KB1):
                nc.sync.dma_start(out=w1_sb[:, e, kb], in_=w1_d[e, :, kb])
            for kb in range(KB2):
                nc.sync.dma_start(out=w2_sb[:, e, kb], in_=w2_d[e, :, kb])

        # HAM warmup: ~4us of back-to-back PE work so the clock gate opens
        # before (and stays open through) the weight-DMA-paced first tiles.
        warm_ps = ps_m2.tile([128, 1024], F32, tag="ps_o")
        for _ in range(16):
            nc.tensor.transpose(warm_ps[:, 0:128], ident, ident)

        # Software pipeline: per-engine instruction queues execute in emission
        # order, so emit x-side work two tiles ahead and h-side one behind:
        #   PE order per step: tx_i, gemm1_i, txh_{i-1}, gemm2_{i-1}
        #   DVE order per step: h_{i-1}, x_{i+2}
        # keeping every engine dense instead of serializing tile-by-tile.
        state = {}

        def emit_x(it):
            """load + quant-dequant (emulates fp8 e4m3fn) for tile `it`.
            scl2 = 2*(max(amax,1e-10)/448); q8 = fp8(x/scl2) equals
            e4m3fn(x/scale)/2 for all normal values."""
            r0 = it * 128
            x_t = xpool.tile([128, H], F32)
            nc.gpsimd.dma_start(out=x_t, in_=xg_d[r0:r0 + 128, :])
            rw_t = rwpool.tile([128, 1], F32)
            nc.gpsimd.dma_start(out=rw_t, in_=rw_d[r0:r0 + 128, :])
            amax = spool.tile([128, KB1], F32, tag="amax_x")
            nc.vector.tensor_reduce(
                out=amax, in_=x_t.rearrange("p (g j) -> p g j", j=128),
                axis=AX, op=OP.max, apply_absolute_value=True)
            scl2 = spool.tile([128, KB1], F32, tag="scl_x")
            nc.vector.tensor_scalar(out=scl2, in0=amax, scalar1=1e-10,
                                    scalar2=2.0 / FP8_MAX, op0=OP.max,
                                    op1=OP.mult)
            inv2 = spool.tile([128, KB1], F32, tag="inv_x")
            nc.vector.reciprocal(inv2, scl2)
            q8 = qpool.tile([128, KB1, 128], FP8, tag="q8_x")
            nc.vector.tensor_tensor(
                out=q8, in0=x_t.rearrange("p (g j) -> p g j", j=128),
                in1=bcast(inv2, 128), op=OP.mult)
            xq = dqpool.tile([128, KB1, 128], TR_DT, tag="xq")
            nc.vector.tensor_tensor(out=xq, in0=q8, in1=bcast(scl2, 128),
                                    op=OP.mult)
            state[it] = {"xq": xq, "rw": rw_t}

        def emit_tx_g1(it):
            """transpose xq + GEMM1 for tile `it`."""
            e = it // tpe
            st = state[it]
            xq = st["xq"]
            xqT = tpool.tile([128, KB1, 128], MM_DT, tag="xqT")
            for half in range(2):
                pst = ps_tx.tile([128, 4, 128], TR_DT, tag="ps_tx")
                for j in range(4):
                    kb = half * 4 + j
                    nc.tensor.transpose(pst[:, j], xq[:, kb, :], ident)
                nc.scalar.copy(out=xqT[:, half * 4:(half + 1) * 4, :], in_=pst)
            ps_gu = ps_m1.tile([128, 1024], F32, tag="ps_gu")
            for kb in range(KB1):
                nc.tensor.matmul(ps_gu[:, 0:512], xqT[:, kb, :],
                                 w1_sb[:, e, kb, 0:512],
                                 start=(kb == 0), stop=(kb == KB1 - 1))
                nc.tensor.matmul(ps_gu[:, 512:1024], xqT[:, kb, :],
                                 w1_sb[:, e, kb, 512:1024],
                                 start=(kb == 0), stop=(kb == KB1 - 1))
            st["gu"] = ps_gu

        def emit_h_g2(it):
            """silu/quant + transpose + GEMM2 + weighted output for tile `it`."""
            e = it // tpe
            r0 = it * 128
            st = state.pop(it)
            ps_gu = st["gu"]
            ps_gate = ps_gu[:, 0:512]
            ps_up = ps_gu[:, 512:1024]
            h_t = hpool.tile([128, I_DIM], F32, tag="h")
            nc.scalar.activation(out=h_t, in_=ps_gate, func=ACT.Silu)
            nc.vector.tensor_tensor(out=h_t, in0=h_t, in1=ps_up, op=OP.mult)
            amax_h = spool.tile([128, KB2], F32, tag="amax_h")
            nc.vector.tensor_reduce(
                out=amax_h, in_=h_t.rearrange("p (g j) -> p g j", j=128),
                axis=AX, op=OP.max, apply_absolute_value=True)
            scl2h = spool.tile([128, KB2], F32, tag="scl_h")
            nc.vector.tensor_scalar(out=scl2h, in0=amax_h, scalar1=1e-10,
                                    scalar2=2.0 / FP8_MAX, op0=OP.max,
                                    op1=OP.mult)
            inv2h = spool.tile([128, KB2], F32, tag="inv_h")
            nc.vector.reciprocal(inv2h, scl2h)
            q8h = qpool.tile([128, KB2, 128], FP8, tag="q8_h")
            nc.vector.tensor_tensor(
                out=q8h, in0=h_t.rearrange("p (g j) -> p g j", j=128),
                in1=bcast(inv2h, 128), op=OP.mult)
            hq = dqpool.tile([128, KB2, 128], BF16, tag="hq")
            nc.gpsimd.tensor_tensor(out=hq, in0=q8h, in1=bcast(scl2h, 128),
                                    op=OP.mult)
            hT = tpool.tile([128, KB2, 128], BF16, tag="hT")
            psh = ps_tx.tile([128, 4, 128], BF16, tag="ps_tx")
            for kb in range(KB2):
                nc.tensor.transpose(psh[:, kb], hq[:, kb, :], ident_bf)
            nc.scalar.copy(out=hT, in_=psh)
            ps_o = ps_m2.tile([128, 1024], F32, tag="ps_o")
            for kb in range(KB2):
                nc.tensor.matmul(ps_o[:, 0:512], hT[:, kb, :],
                                 w2_sb[:, e, kb, 0:512],
                                 start=(kb == 0), stop=(kb == KB2 - 1))
                nc.tensor.matmul(ps_o[:, 512:1024], hT[:, kb, :],
                                 w2_sb[:, e, kb, 512:1024],
                                 start=(kb == 0), stop=(kb == KB2 - 1))
            o_t = opool.tile([128, H], F32, tag="o")
            nc.scalar.activation(out=o_t[:, 0:512], in_=ps_o[:, 0:512],
                                 func=ACT.Copy, scale=st["rw"])
            nc.scalar.activation(out=o_t[:, 512:1024], in_=ps_o[:, 512:1024],
                                 func=ACT.Copy, scale=st["rw"])
            nc.gpsimd.dma_start(out=out_d[r0:r0 + 128, :], in_=o_t)

        for j in range(min(3, ntiles)):
            emit_x(j)
        for it in range(ntiles):
            emit_tx_g1(it)
            if it >= 1:
                emit_h_g2(it - 1)
            if it + 3 < ntiles:
                emit_x(it + 3)
        emit_h_g2(ntiles - 1)

    if do_compile:
        nc.compile()
    return nc


def _get_program(cap: int):
    key = (cap, MM_DT_NAME)
    if key not in _PROGRAM_CACHE:
        _PROGRAM_CACHE[key] = _build_program(cap)
    return _PROGRAM_CACHE[key]


def _dequant_weight(w, s, g=G):
    E_, O_, K_ = w.shape
    wb = w.reshape(E_, O_ // g, g, K_ // g, g)
    return (wb * s[:, :, None, :, None]).reshape(E_, O_, K_)


def _prep(inputs):
    hs = np.ascontiguousarray(np.asarray(inputs["hidden_states"], np.float32))
    idx = np.asarray(inputs["top_k_index"]).astype(np.int64)
    tkw = np.asarray(inputs["top_k_weights"], np.float32)
    gup = np.asarray(inputs["gate_up_proj"], np.float32)
    gup_s = np.asarray(inputs["gate_up_proj_scale_inv"], np.float32)
    dn = np.asarray(inputs["down_proj"], np.float32)
    dn_s = np.asarray(inputs["down_proj_scale_inv"], np.float32)

    # routing tables: merge duplicate (token, expert) pairs (the reference
    # sums top-k weights per expert), then group by expert
    flat_e = idx.reshape(-1)
    flat_t = np.repeat(np.arange(T, dtype=np.int64), TK)
    flat_w = tkw.reshape(-1).astype(np.float64)
    key = flat_e * T + flat_t
    uk, inv = np.unique(key, return_inverse=True)
    sw = np.bincount(inv, weights=flat_w).astype(np.float32)
    se = (uk // T).astype(np.int64)
    st = (uk % T).astype(np.int64)
    counts = np.bincount(se, minlength=E)
    cap = int(np.ceil(max(int(counts.max()), 1) / 128.0) * 128)
    R = E_LOC * cap

    starts = np.zeros(E + 1, np.int64)
    np.cumsum(counts, out=starts[1:])

    # exact f32 dequant (same arithmetic as the reference), -> [K, O] layout
    w1_full = _dequant_weight(gup, gup_s)            # [E, O1, H]
    w2_full = _dequant_weight(dn, dn_s)              # [E, O2, I]
    mmdt = ml_dtypes.bfloat16 if MM_DT_NAME == "bfloat16" else np.float32
    # [E, K, O] -> [E, 128, KB, O]
    w1_t = np.ascontiguousarray(
        w1_full.transpose(0, 2, 1).reshape(E, KB1, 128, O1).transpose(0, 2, 1, 3)
    ).astype(mmdt)
    w2_t = np.ascontiguousarray(
        w2_full.transpose(0, 2, 1).reshape(E, KB2, 128, O2).transpose(0, 2, 1, 3)
    ).astype(ml_dtypes.bfloat16)

    in_maps = []
    tok_core = []      # per-core valid token ids (concatenated per expert)
    nvalid_core = []   # per-core list of (row_offset, count)
    for c in range(NCORES):
        rows_idx = np.zeros(R, np.int64)
        rw_vec = np.zeros(R, np.float32)
        segs = []
        for j in range(E_LOC):
            e = c * E_LOC + j
            n = int(counts[e])
            s0, r0 = starts[e], j * cap
            rows_idx[r0:r0 + n] = st[s0:s0 + n]
            rw_vec[r0:r0 + n] = sw[s0:s0 + n]
            segs.append((r0, n))
        xg = hs[rows_idx]  # [R, H]
        in_maps.append({
            "xg": xg,
            "rw": rw_vec.reshape(R, 1),
            "w1": np.ascontiguousarray(w1_t[c * E_LOC:(c + 1) * E_LOC]),
            "w2": np.ascontiguousarray(w2_t[c * E_LOC:(c + 1) * E_LOC]),
        })
        tok_core.append(rows_idx)
        nvalid_core.append(segs)
    return cap, in_maps, tok_core, nvalid_core


def _combine(results, tok_core, nvalid_core):
    out = np.zeros((T, H), np.float32)
    for c in range(NCORES):
        res = results[c]["out"]
        for (r0, n) in nvalid_core[c]:
            if n:
                np.add.at(out, tok_core[c][r0:r0 + n], res[r0:r0 + n])
    return out


def kernel_with_results(inputs, trace=False):
    from concourse.bass_utils import run_bass_kernel_spmd
    cap, in_maps, tok_core, nvalid_core = _prep(inputs)
    nc = _get_program(cap)
    bres = run_bass_kernel_spmd(nc, in_maps, core_ids=list(range(NCORES)),
                                trace=trace)
    out = _combine(bres.results, tok_core, nvalid_core)
    return out, bres


def kernel(**inputs) -> np.ndarray:
    out, _ = kernel_with_results(inputs, trace=False)
    return out
